# revision 44
# baseline (speedup 1.0000x reference)
"""Bass/Tile TRN2 kernel for nn_BertSelfAttention2 (B=2, S=2048, D=1024, H=16).

Sharding: 8 cores = 2 (batch) x 4 (head groups of 4 heads). Each core
computes Q/K projections for its 4 heads (2 packed pairs), the modified
attention (kt = softplus(k), v = q + k, mask on the query axis), and writes
its [*, 256] slice of the output.

Key tricks vs the naive version:
- Softmax over keys is invariant to a permutation of the sequence axis, so
  the host permutes each batch's sequence to put unmasked queries first.
  Only the first NQ (= n_unmasked rounded up to 64) query columns run
  through attention; every masked query's reference output is the SAME
  uniform average of V, which the device returns as a single [256] vector
  (vout) that the host broadcasts. This deletes the mask logic entirely and
  skips ~40% of scores/exp/ctx work.
- Everything is computed in "T" orientation (scoresT[k, q]); all matmuls in
  bf16 (1 cycle/row); q is pre-scaled by 16*log2(e) so the scores psum is
  directly the bf16 exponent-bits scale.
- exp() is split across three engines: ACT runs the real Exp (bf16 out);
  DVE and Pool synthesize bf16 bits directly via the Schraudolph trick
  (bits = psum + K16, clamped at 0, written as uint16, bitcast to bf16).
  A constant bits offset is a constant multiplicative factor that softmax
  normalization cancels, so the two paths mix safely (K16 calibrated).
- kt = softplus(k) computed as Ln(Exp(k)+1) so every ACT op stays in the
  'natural_log_exp_and_others' table set (no 1.3us table reloads).
"""
import sys

if "/opt/trn_rl_repo" not in sys.path:
    sys.path.insert(0, "/opt/trn_rl_repo")

import numpy as np

B, S, D = 2, 2048, 1024
H = 16
HD = 64
NCORES = 8
HPC = H // (NCORES // B)     # heads per core = 4
NG = HPC // 2                # head-pair groups per core = 2
SC = 4                       # 512-wide seq chunks for the projection
C16 = float(16.0 * np.log2(np.e))        # q pre-scale: psum = C16 * score
SHIFT = -1.0                              # exp(s/8 + SHIFT), cancels in softmax
CCAL = -5.0                               # bits-exp calibration
K16 = 16256.0 + 128.0 * np.log2(np.e) * SHIFT + CCAL
EXP_SCALE = float(1.0 / (8.0 * C16))      # ACT: Exp(psum*EXP_SCALE + SHIFT)

_CACHE = {}


def _build(NQ):
    import concourse.tile as tile
    from concourse import bacc, mybir
    from concourse.masks import make_identity

    F32 = mybir.dt.float32
    F32R = mybir.dt.float32r
    BF16 = mybir.dt.bfloat16
    U16 = mybir.dt.uint16
    AF = mybir.ActivationFunctionType
    ALU = mybir.AluOpType

    # window chunks (aligned to the 512-wide proj chunks)
    WCS = []
    base = 0
    while base < NQ:
        WCS.append((base, min(512, NQ - base)))
        base += 512
    KC = S // 128            # 16 key chunks

    nc = bacc.Bacc(None, target_bir_lowering=False, debug=False)

    # fp8/bf16 payloads are shipped packed in f32 dram tensors (the PJRT
    # input path silently zero-fills non-f32 inputs); tiles bitcast on DMA.
    # Projection runs as a 3-pass fp8 hi+lo residual (W pre-scaled by 32 on
    # the host to stay out of e4m3 subnormals) with DoubleRow matmuls.
    xh = nc.declare_dram_parameter("xh", [SC * 4 * 128, 256], F32, isOutput=False)
    xl = nc.declare_dram_parameter("xl", [SC * 4 * 128, 256], F32, isOutput=False)
    wqh = nc.declare_dram_parameter("wqh", [NG * 4 * 128, 64], F32, isOutput=False)
    wql = nc.declare_dram_parameter("wql", [NG * 4 * 128, 64], F32, isOutput=False)
    wkh = nc.declare_dram_parameter("wkh", [NG * 4 * 128, 64], F32, isOutput=False)
    wkl = nc.declare_dram_parameter("wkl", [NG * 4 * 128, 64], F32, isOutput=False)
    bqs = nc.declare_dram_parameter("bqs", [NG * 128], F32, isOutput=False)
    bqk = nc.declare_dram_parameter("bqk", [NG * 128], F32, isOutput=False)
    bkb = nc.declare_dram_parameter("bkb", [NG * 128], F32, isOutput=False)
    out = nc.declare_dram_parameter("out", [NQ, NG * 128], F32, isOutput=True)
    vout = nc.declare_dram_parameter("vout", [NG * 128, 1], F32, isOutput=True)

    with tile.TileContext(nc) as tc:
        with tc.tile_pool(name="consts", bufs=1) as consts, \
             tc.tile_pool(name="big", bufs=1) as big, \
             tc.tile_pool(name="tmp", bufs=2) as tmp, \
             tc.tile_pool(name="expp", bufs=3) as expp, \
             tc.tile_pool(name="ep", bufs=2) as ep, \
             tc.tile_pool(name="ps_m", bufs=3, space="PSUM") as ps_m, \
             tc.tile_pool(name="ps_s", bufs=3, space="PSUM") as ps_s, \
             tc.tile_pool(name="ps_c", bufs=1, space="PSUM") as ps_c:

            ident = consts.tile([128, 128], BF16)
            make_identity(nc, ident)
            ident_f = consts.tile([65, 65], F32)
            make_identity(nc, ident_f)
            shift_t = consts.tile([128, 1], F32)
            nc.gpsimd.memset(shift_t, SHIFT)

            # weights for group 0 + biases first so proj can start ASAP
            FP8 = mybir.dt.float8e4
            PM = mybir.MatmulPerfMode.DoubleRow
            w_t = {}
            for nm in ("wqh", "wql", "wkh", "wkl"):
                w_t[nm] = [[consts.tile([128, 2, 128], FP8,
                                        tag=f"{nm}{g}_{dcp}",
                                        name=f"{nm}{g}_{dcp}")
                            for dcp in range(4)] for g in range(NG)]
            bqs_t, bqk_t, bkb_t = [], [], []
            for g in range(NG):
                for lst, par, nm in ((bqs_t, bqs, "bqs"), (bqk_t, bqk, "bqk"),
                                     (bkb_t, bkb, "bkb")):
                    t = consts.tile([128, 1], F32, tag=f"{nm}{g}",
                                    name=f"{nm}{g}")
                    nc.scalar.dma_start(
                        out=t,
                        in_=par[g * 128:(g + 1) * 128].rearrange(
                            "(p o) -> p o", o=1))
                    lst.append(t)

            for dcp in range(4):
                nc.scalar.dma_start(out=w_t["wqh"][0][dcp].bitcast(F32),
                                    in_=wqh[dcp * 128:(dcp + 1) * 128, :])
                nc.scalar.dma_start(out=w_t["wql"][0][dcp].bitcast(F32),
                                    in_=wql[dcp * 128:(dcp + 1) * 128, :])
                nc.gpsimd.dma_start(out=w_t["wkh"][0][dcp].bitcast(F32),
                                    in_=wkh[dcp * 128:(dcp + 1) * 128, :])
                nc.gpsimd.dma_start(out=w_t["wkl"][0][dcp].bitcast(F32),
                                    in_=wkl[dcp * 128:(dcp + 1) * 128, :])

            # X^T hi/lo tiles, sc-major so the first proj chunk is ready early
            xh_t = [[big.tile([128, 2, 512], FP8, tag=f"xh{dcp}_{sc}",
                              name=f"xh{dcp}_{sc}") for sc in range(SC)]
                    for dcp in range(4)]
            xl_t = [[big.tile([128, 2, 512], FP8, tag=f"xl{dcp}_{sc}",
                              name=f"xl{dcp}_{sc}") for sc in range(SC)]
                    for dcp in range(4)]
            for sc in range(SC):
                for dcp in range(4):
                    base = (sc * 4 + dcp) * 128
                    nc.sync.dma_start(out=xh_t[dcp][sc].bitcast(F32),
                                      in_=xh[base:base + 128, :])
                    nc.sync.dma_start(out=xl_t[dcp][sc].bitcast(F32),
                                      in_=xl[base:base + 128, :])

            for dcp in range(4):
                base = (4 + dcp) * 128
                nc.gpsimd.dma_start(out=w_t["wqh"][1][dcp].bitcast(F32),
                                    in_=wqh[base:base + 128, :])
                nc.gpsimd.dma_start(out=w_t["wql"][1][dcp].bitcast(F32),
                                    in_=wql[base:base + 128, :])
                nc.gpsimd.dma_start(out=w_t["wkh"][1][dcp].bitcast(F32),
                                    in_=wkh[base:base + 128, :])
                nc.gpsimd.dma_start(out=w_t["wkl"][1][dcp].bitcast(F32),
                                    in_=wkl[base:base + 128, :])

            # persistent activations
            # qtp: zero-padded per head so scores run with full K=128
            qtp = [[[big.tile([128, w], BF16, tag=f"qtp{g}_{hh}_{wi}",
                              name=f"qtp{g}_{hh}_{wi}")
                     for wi, (b0, w) in enumerate(WCS)]
                    for hh in range(2)] for g in range(NG)]
            kt = [big.tile([128, S], BF16, tag=f"kt{g}", name=f"kt{g}")
                  for g in range(NG)]
            tep = [big.tile([128, S], F32, tag=f"te{g}", name=f"te{g}")
                   for g in range(NG)]
            vp = [[big.tile([128, 65], BF16, tag=f"vp{h}_{kc}",
                            name=f"vp{h}_{kc}") for kc in range(KC)]
                  for h in range(HPC)]
            for g in range(NG):
                for wi in range(len(WCS)):
                    nc.gpsimd.memset(qtp[g][0][wi][64:128, :], 0.0)
                    nc.gpsimd.memset(qtp[g][1][wi][0:64, :], 0.0)
            for h in range(HPC):
                for kc in range(KC):
                    nc.gpsimd.memset(vp[h][kc][:, 64:65], 1.0)

            ones_t = consts.tile([128, 1], BF16)
            nc.gpsimd.memset(ones_t, 1.0)

            def emit_vtrans(g, sc, vts):
                for hh in range(2):
                    h = g * 2 + hh
                    hsl = slice(hh * 64, (hh + 1) * 64)
                    for j in range(4):
                        kc = sc * 4 + j
                        pv = ps_m.tile([128, 64], BF16, tag="ep",
                                       name=f"pv{g}_{hh}_{kc}")
                        nc.tensor.transpose(pv, vts[hsl, j * 128:(j + 1) * 128],
                                            ident[hsl, hsl])
                        nc.vector.tensor_copy(vp[h][kc][:, 0:64], pv)

            def proj_group(g):
                vts_hist = []
                for sc in range(SC):
                    pq = ps_m.tile([128, 512], F32, tag="ep", name=f"pq{g}_{sc}")
                    mm = 0
                    for wn, xt_ in (("wqh", xh_t), ("wqh", xl_t),
                                    ("wql", xh_t)):
                        for dcp in range(4):
                            nc.tensor.matmul(pq, w_t[wn][g][dcp],
                                             xt_[dcp][sc], perf_mode=PM,
                                             start=(mm == 0), stop=(mm == 11))
                            mm += 1
                    pk = ps_m.tile([128, 512], F32, tag="ep", name=f"pk{g}_{sc}")
                    mm = 0
                    for wn, xt_ in (("wkh", xh_t), ("wkh", xl_t),
                                    ("wkl", xh_t)):
                        for dcp in range(4):
                            nc.tensor.matmul(pk, w_t[wn][g][dcp],
                                             xt_[dcp][sc], perf_mode=PM,
                                             start=(mm == 0), stop=(mm == 11))
                            mm += 1
                    # psum holds 32*(proj); /32 is folded into every consumer
                    # v = q + k + (bq + bk), bf16.
                    # (DVE cannot read two PSUM operands in one instruction.)
                    tqv = tmp.tile([128, 512], F32, tag="tqv",
                                   name=f"tqv{g}_{sc}")
                    nc.vector.tensor_scalar(tqv, pq, 1.0 / 32, bqk_t[g],
                                            ALU.mult, ALU.add)
                    vts = tmp.tile([128, 512], BF16, tag="vts",
                                   name=f"vts{g}_{sc}")
                    nc.vector.scalar_tensor_tensor(
                        out=vts, in0=pk, scalar=1.0 / 32, in1=tqv,
                        op0=ALU.mult, op1=ALU.add)
                    # softplus part 1: exp(k + bk) into one per-group tile;
                    # the single whole-tile Ln below cannot be interleaved by
                    # the scheduler, so the ACT table set switches only twice
                    # per group
                    nc.scalar.activation(out=tep[g][:, sc * 512:(sc + 1) * 512],
                                         in_=pk, func=AF.Exp, scale=1.0 / 32,
                                         bias=bkb_t[g])
                    # q, scaled for the exp trick (window chunks only); on
                    # DVE — an ACT Identity would thrash the ACT table set
                    if sc < len(WCS):
                        w = WCS[sc][1]
                        nc.vector.tensor_scalar(qtp[g][0][sc][0:64, :],
                                                pq[0:64, 0:w], C16 / 32,
                                                bqs_t[g][0:64],
                                                ALU.mult, ALU.add)
                        nc.vector.tensor_scalar(qtp[g][1][sc][64:128, :],
                                                pq[64:128, 0:w], C16 / 32,
                                                bqs_t[g][64:128],
                                                ALU.mult, ALU.add)
                    vts_hist.append(vts)
                    if sc > 0:
                        emit_vtrans(g, sc - 1, vts_hist[sc - 1])
                    if sc == SC - 1:
                        emit_vtrans(g, sc, vts_hist[sc])
                nc.scalar.activation(out=kt[g], in_=tep[g], func=AF.Ln,
                                     bias=1.0)

            # exp engine rotation (Pool cannot read PSUM): ACT 60%, DVE 40%
            _exp_i = [0]
            _pat = ("act", "dve", "act", "act", "dve")

            def emit_exp(e_out, s_in):
                kind = _pat[_exp_i[0] % len(_pat)]
                _exp_i[0] += 1
                if kind == "act":
                    nc.scalar.activation(out=e_out, in_=s_in, func=AF.Exp,
                                         scale=EXP_SCALE, bias=shift_t)
                else:
                    nc.vector.tensor_scalar(e_out.bitcast(U16), s_in, K16, 0.0,
                                            ALU.add, ALU.max)

            def attn_group(g):
                vpA = vp[g * 2]
                vpB = vp[g * 2 + 1]
                # vsum (uniform masked-query row): ones-matmul over the
                # transposed v tiles, summed across keys on the PE
                for hh in range(2):
                    h = g * 2 + hh
                    pvs = ps_c.tile([65, 1], F32, tag=("cA" if hh == 0
                                                       else "cB"),
                                    name=f"pvs{g}_{hh}")
                    for kc in range(KC):
                        nc.tensor.matmul(pvs, vp[h][kc], ones_t,
                                         start=(kc == 0), stop=(kc == KC - 1))
                    vs_sb = ep.tile([64, 1], F32, tag="vss",
                                    name=f"vss{g}_{hh}")
                    nc.vector.tensor_copy(vs_sb, pvs[0:64, :])
                    row = g * 128 + hh * 64
                    nc.sync.dma_start(out=vout[row:row + 64, :], in_=vs_sb)
                for wi, (wb, w) in enumerate(WCS):
                    qA = qtp[g][0][wi]
                    qB = qtp[g][1][wi]
                    cA = ps_c.tile([65, w], F32, tag="cA", name=f"cA{g}_{wi}")
                    cB = ps_c.tile([65, w], F32, tag="cB", name=f"cB{g}_{wi}")
                    for kc in range(KC):
                        lhs = kt[g][:, kc * 128:(kc + 1) * 128]
                        sA = ps_s.tile([128, w], F32, tag="s",
                                       name=f"sA{g}_{wi}_{kc}")
                        nc.tensor.matmul(sA, lhs, qA, start=True, stop=True)
                        sB = ps_s.tile([128, w], F32, tag="s",
                                       name=f"sB{g}_{wi}_{kc}")
                        nc.tensor.matmul(sB, lhs, qB, start=True, stop=True)
                        eA = expp.tile([128, w], BF16, tag="eA",
                                       name=f"eA{g}_{wi}_{kc}")
                        emit_exp(eA, sA)
                        eB = expp.tile([128, w], BF16, tag="eB",
                                       name=f"eB{g}_{wi}_{kc}")
                        emit_exp(eB, sB)
                        nc.tensor.matmul(cA, vpA[kc], eA,
                                         start=(kc == 0), stop=(kc == KC - 1))
                        nc.tensor.matmul(cB, vpB[kc], eB,
                                         start=(kc == 0), stop=(kc == KC - 1))
                    # epilogue: transpose back, normalize, store
                    csA = ep.tile([65, w], F32, tag="csA", name=f"csA{g}_{wi}")
                    nc.vector.tensor_copy(csA, cA)
                    csB = ep.tile([65, w], F32, tag="csB", name=f"csB{g}_{wi}")
                    nc.vector.tensor_copy(csB, cB)
                    jb = 0
                    while jb < w:
                        jw = min(128, w - jb)
                        ptA = ps_m.tile([jw, 65], F32, tag="ep",
                                        name=f"ptA{g}_{wi}_{jb}")
                        nc.tensor.transpose(ptA, csA[:, jb:jb + jw],
                                            ident_f)
                        ptB = ps_m.tile([jw, 65], F32, tag="ep",
                                        name=f"ptB{g}_{wi}_{jb}")
                        nc.tensor.transpose(ptB, csB[:, jb:jb + jw],
                                            ident_f)
                        rA = ep.tile([jw, 1], F32, tag="rA",
                                     name=f"rA{g}_{wi}_{jb}")
                        nc.vector.reciprocal(rA, ptA[:, 64:65])
                        rB = ep.tile([jw, 1], F32, tag="rB",
                                     name=f"rB{g}_{wi}_{jb}")
                        nc.vector.reciprocal(rB, ptB[:, 64:65])
                        cf = ep.tile([jw, 128], F32, tag="cf",
                                     name=f"cf{g}_{wi}_{jb}")
                        nc.vector.tensor_scalar_mul(cf[:, 0:64], ptA[:, 0:64], rA)
                        nc.vector.tensor_scalar_mul(cf[:, 64:128], ptB[:, 0:64],
                                                    rB)
                        row = wb + jb
                        nc.gpsimd.dma_start(
                            out=out[row:row + jw, g * 128:(g + 1) * 128],
                            in_=cf)
                        jb += jw

            for g in range(NG):
                proj_group(g)
                attn_group(g)

    nc.finalize()
    return nc


def _get_nc(NQ):
    key = ("nc", NQ)
    if key not in _CACHE:
        _CACHE[key] = _build(NQ)
    return _CACHE[key]


def _shard_inputs(hidden_states, attention_mask, Wq, bq, Wk, bk):
    import ml_dtypes
    BF = ml_dtypes.bfloat16

    hs = np.asarray(hidden_states, dtype=np.float32)
    am = np.asarray(attention_mask)
    Wq = np.asarray(Wq, dtype=np.float32)
    Wk = np.asarray(Wk, dtype=np.float32)
    bq = np.asarray(bq, dtype=np.float32)
    bk = np.asarray(bk, dtype=np.float32)

    perms, n_ums = [], []
    for b in range(B):
        perm = np.argsort(-(am[b] != 0).astype(np.int32), kind="stable")
        perms.append(perm)
        n_ums.append(int((am[b] != 0).sum()))
    NQ = max(128, -(-max(n_ums) // 64) * 64)
    NQ = min(NQ, S)
    _CACHE["meta"] = {"perms": perms, "n_ums": n_ums, "NQ": NQ}

    import ml_dtypes as _mld
    E4 = _mld.float8_e4m3

    def _pack_x(xT):
        # DR tiles (sc, dcp): [128, 2, 512]
        return np.ascontiguousarray(
            xT.reshape(4, 2, 128, SC, 512).transpose(3, 0, 2, 1, 4)
            .reshape(SC * 4 * 128, 1024))

    xhs, xls = [], []
    for b in range(B):
        xT = np.ascontiguousarray(hs[b][perms[b]].T)
        x_hi = xT.astype(E4)
        x_lo = (xT - x_hi.astype(np.float32)).astype(E4)
        xhs.append(_pack_x(x_hi.astype(np.float32)).astype(E4)
                   .view(np.float32))
        xls.append(_pack_x(x_lo.astype(np.float32)).astype(E4)
                   .view(np.float32))

    in_maps = []
    for c in range(NCORES):
        b = c // (NCORES // B)
        hg = c % (NCORES // B)
        cols = slice(hg * 2 * 128, (hg + 1) * 2 * 128)

        def _tile_w(Wp):
            # DR tiles (g, dcp): [128, 2, 128]
            return np.ascontiguousarray(
                Wp.reshape(4, 2, 128, NG, 128).transpose(3, 0, 2, 1, 4)
                .reshape(NG * 4 * 128, 256)).astype(E4).view(np.float32)

        def _hilo(W):
            Ws = (32.0 * W[:, cols]).astype(np.float32)
            w_hi = Ws.astype(E4)
            w_lo = (Ws - w_hi.astype(np.float32)).astype(E4)
            return (_tile_w(w_hi.astype(np.float32)),
                    _tile_w(w_lo.astype(np.float32)))

        wqh_, wql_ = _hilo(Wq)
        wkh_, wkl_ = _hilo(Wk)
        in_maps.append({
            "xh": xhs[b],
            "xl": xls[b],
            "wqh": wqh_, "wql": wql_, "wkh": wkh_, "wkl": wkl_,
            "bqs": np.ascontiguousarray(C16 * bq[cols]).astype(np.float32),
            "bqk": np.ascontiguousarray(bq[cols] + bk[cols]).astype(np.float32),
            "bkb": np.ascontiguousarray(bk[cols]).astype(np.float32),
        })
    return in_maps


def _gather(results):
    meta = _CACHE["meta"]
    perms, n_ums, NQ = meta["perms"], meta["n_ums"], meta["NQ"]
    full = np.empty((B, S, D), dtype=np.float32)
    for c in range(NCORES):
        b = c // (NCORES // B)
        hg = c % (NCORES // B)
        cols = slice(hg * 2 * 128, (hg + 1) * 2 * 128)
        permuted = np.empty((S, 2 * 128), dtype=np.float32)
        permuted[:NQ] = results[c]["out"]
        permuted[n_ums[b]:] = (results[c]["vout"][:, 0] / np.float32(S))[None, :]
        full[b, :, cols] = permuted[np.argsort(perms[b])]
    return full


def run_sharded(in_maps, **kw):
    from concourse.bass_utils import run_bass_kernel_spmd
    nc = _get_nc(_CACHE["meta"]["NQ"])
    return run_bass_kernel_spmd(nc, in_maps, list(range(NCORES)), **kw)


def kernel(hidden_states, attention_mask, Wq, bq, Wk, bk):
    in_maps = _shard_inputs(hidden_states, attention_mask, Wq, bq, Wk, bk)
    res = run_sharded(in_maps)
    return _gather(res.results)


# revision 58
# speedup vs baseline: 1.2498x; 1.2498x over previous
"""Bass/Tile TRN2 kernel for nn_BertSelfAttention2 (B=2, S=2048, D=1024, H=16).

Sharding: 8 cores = 2 (batch) x 4 (head groups of 4 heads). Each core
computes Q/K projections for its 4 heads (2 packed pairs), the modified
attention (kt = softplus(k), v = q + k, mask on the query axis), and writes
its [*, 256] slice of the output.

Key tricks vs the naive version:
- Softmax over keys is invariant to a permutation of the sequence axis, so
  the host permutes each batch's sequence to put unmasked queries first.
  Only the first NQ (= n_unmasked rounded up to 64) query columns run
  through attention; every masked query's reference output is the SAME
  uniform average of V, which the device returns as a single [256] vector
  (vout) that the host broadcasts. This deletes the mask logic entirely and
  skips ~40% of scores/exp/ctx work.
- Everything is computed in "T" orientation (scoresT[k, q]); all matmuls in
  bf16 (1 cycle/row); q is pre-scaled by 16*log2(e) so the scores psum is
  directly the bf16 exponent-bits scale.
- exp() is split across three engines: ACT runs the real Exp (bf16 out);
  DVE and Pool synthesize bf16 bits directly via the Schraudolph trick
  (bits = psum + K16, clamped at 0, written as uint16, bitcast to bf16).
  A constant bits offset is a constant multiplicative factor that softmax
  normalization cancels, so the two paths mix safely (K16 calibrated).
- kt = softplus(k) computed as Ln(Exp(k)+1) so every ACT op stays in the
  'natural_log_exp_and_others' table set (no 1.3us table reloads).
"""
import sys

if "/opt/trn_rl_repo" not in sys.path:
    sys.path.insert(0, "/opt/trn_rl_repo")

import numpy as np

B, S, D = 2, 2048, 1024
H = 16
HD = 64
NCORES = 8
HPC = H // (NCORES // B)     # heads per core = 4
NG = HPC // 2                # head-pair groups per core = 2
SC = 4                       # 512-wide seq chunks for the projection
C16 = float(16.0 * np.log2(np.e))        # q pre-scale: psum = C16 * score
SHIFT = -1.0                              # exp(s/8 + SHIFT), cancels in softmax
CCAL = -5.0                               # bits-exp calibration
K16 = 16256.0 + 128.0 * np.log2(np.e) * SHIFT + CCAL
EXP_SCALE = float(1.0 / (8.0 * C16))      # ACT: Exp(psum*EXP_SCALE + SHIFT)

_CACHE = {}


def _build(NQ):
    import concourse.tile as tile
    from concourse import bacc, mybir
    from concourse.masks import make_identity

    F32 = mybir.dt.float32
    F32R = mybir.dt.float32r
    BF16 = mybir.dt.bfloat16
    U16 = mybir.dt.uint16
    AF = mybir.ActivationFunctionType
    ALU = mybir.AluOpType

    # window chunks: 256 wide so a double-wide (2-head) scores matmul output
    # [128, 2*w] fits one PSUM bank
    WCS = []
    base = 0
    while base < NQ:
        WCS.append((base, min(256, NQ - base)))
        base += 256
    KC = S // 128            # 16 key chunks

    nc = bacc.Bacc(None, target_bir_lowering=False, debug=False)

    # bf16 payloads are shipped packed in f32 dram tensors (the PJRT input
    # path silently zero-fills non-f32 inputs); tiles bitcast on DMA.
    xt = nc.declare_dram_parameter("xt", [SC * 8 * 128, 256], F32, isOutput=False)
    wq = nc.declare_dram_parameter("wq", [NG * 8 * 128, 64], F32, isOutput=False)
    wk = nc.declare_dram_parameter("wk", [NG * 8 * 128, 64], F32, isOutput=False)
    bqs = nc.declare_dram_parameter("bqs", [NG * 128], F32, isOutput=False)
    bqk = nc.declare_dram_parameter("bqk", [NG * 128], F32, isOutput=False)
    bkb = nc.declare_dram_parameter("bkb", [NG * 128], F32, isOutput=False)
    out = nc.declare_dram_parameter("out", [NQ, NG * 128], F32, isOutput=True)
    vout = nc.declare_dram_parameter("vout", [NG * 128, 1], F32, isOutput=True)

    with tile.TileContext(nc) as tc:
        with tc.tile_pool(name="consts", bufs=1) as consts, \
             tc.tile_pool(name="big", bufs=1) as big, \
             tc.tile_pool(name="tmp", bufs=2) as tmp, \
             tc.tile_pool(name="expp", bufs=3) as expp, \
             tc.tile_pool(name="ep", bufs=2) as ep, \
             tc.tile_pool(name="ps_m", bufs=3, space="PSUM") as ps_m, \
             tc.tile_pool(name="ps_s", bufs=3, space="PSUM") as ps_s, \
             tc.tile_pool(name="ps_c", bufs=1, space="PSUM") as ps_c:

            ident = consts.tile([128, 128], BF16)
            make_identity(nc, ident)
            ident_f = consts.tile([65, 65], F32)
            make_identity(nc, ident_f)
            shift_t = consts.tile([128, 1], F32)
            nc.gpsimd.memset(shift_t, SHIFT)

            # weights for group 0 + biases first so proj can start ASAP
            wq_t = [[consts.tile([128, 128], BF16, tag=f"wq{g}_{dc}",
                                 name=f"wq{g}_{dc}") for dc in range(8)]
                    for g in range(NG)]
            wk_t = [[consts.tile([128, 128], BF16, tag=f"wk{g}_{dc}",
                                 name=f"wk{g}_{dc}") for dc in range(8)]
                    for g in range(NG)]
            bqs_t, bqk_t, bkb_t = [], [], []
            for g in range(NG):
                for lst, par, nm in ((bqs_t, bqs, "bqs"), (bqk_t, bqk, "bqk"),
                                     (bkb_t, bkb, "bkb")):
                    t = consts.tile([128, 1], F32, tag=f"{nm}{g}",
                                    name=f"{nm}{g}")
                    nc.scalar.dma_start(
                        out=t,
                        in_=par[g * 128:(g + 1) * 128].rearrange(
                            "(p o) -> p o", o=1))
                    lst.append(t)

            for dc in range(8):
                nc.scalar.dma_start(out=wq_t[0][dc].bitcast(F32),
                                    in_=wq[dc * 128:(dc + 1) * 128, :])
                nc.gpsimd.dma_start(out=wk_t[0][dc].bitcast(F32),
                                    in_=wk[dc * 128:(dc + 1) * 128, :])

            # X^T tiles, sc-major so the first projection chunk is ready early
            xt_t = [[big.tile([128, 512], BF16, tag=f"xt{dc}_{sc}",
                              name=f"xt{dc}_{sc}") for sc in range(SC)]
                    for dc in range(8)]
            for sc in range(SC):
                for dc in range(8):
                    base = (sc * 8 + dc) * 128
                    nc.sync.dma_start(out=xt_t[dc][sc].bitcast(F32),
                                      in_=xt[base:base + 128, :])

            for dc in range(8):
                base = (8 + dc) * 128
                nc.gpsimd.dma_start(out=wq_t[1][dc].bitcast(F32),
                                    in_=wq[base:base + 128, :])
                nc.gpsimd.dma_start(out=wk_t[1][dc].bitcast(F32),
                                    in_=wk[base:base + 128, :])

            # persistent activations
            # qtp: both heads of a pair in one [128, 2, w] tile, zero-padded
            # so scores for the pair run as ONE double-wide K=128 matmul
            qtp = [[big.tile([128, 2, w], BF16, tag=f"qtp{g}_{wi}",
                             name=f"qtp{g}_{wi}")
                    for wi, (b0, w) in enumerate(WCS)] for g in range(NG)]
            kt = [big.tile([128, S], BF16, tag=f"kt{g}", name=f"kt{g}")
                  for g in range(NG)]
            tep = [big.tile([128, S], F32, tag=f"te{g}", name=f"te{g}")
                   for g in range(NG)]
            vp = [[big.tile([128, 65], BF16, tag=f"vp{h}_{kc}",
                            name=f"vp{h}_{kc}") for kc in range(KC)]
                  for h in range(HPC)]
            for g in range(NG):
                for wi in range(len(WCS)):
                    nc.gpsimd.memset(qtp[g][wi][64:128, 0, :], 0.0)
                    nc.gpsimd.memset(qtp[g][wi][0:64, 1, :], 0.0)
            for h in range(HPC):
                for kc in range(KC):
                    nc.gpsimd.memset(vp[h][kc][:, 64:65], 1.0)

            ones_t = consts.tile([128, 1], BF16)
            nc.gpsimd.memset(ones_t, 1.0)

            def emit_vtrans(g, sc, vts):
                for hh in range(2):
                    h = g * 2 + hh
                    hsl = slice(hh * 64, (hh + 1) * 64)
                    for j in range(4):
                        kc = sc * 4 + j
                        pv = ps_m.tile([128, 64], BF16, tag="ep",
                                       name=f"pv{g}_{hh}_{kc}")
                        nc.tensor.transpose(pv, vts[hsl, j * 128:(j + 1) * 128],
                                            ident[hsl, hsl])
                        nc.vector.tensor_copy(vp[h][kc][:, 0:64], pv)

            def proj_group(g):
                vts_hist = []
                for sc in range(SC):
                    pq = ps_m.tile([128, 512], F32, tag="ep", name=f"pq{g}_{sc}")
                    for dc in range(8):
                        nc.tensor.matmul(pq, wq_t[g][dc], xt_t[dc][sc],
                                         start=(dc == 0), stop=(dc == 7))
                    pk = ps_m.tile([128, 512], F32, tag="ep", name=f"pk{g}_{sc}")
                    for dc in range(8):
                        nc.tensor.matmul(pk, wk_t[g][dc], xt_t[dc][sc],
                                         start=(dc == 0), stop=(dc == 7))
                    # v = q + k + (bq + bk), bf16.
                    # (DVE cannot read two PSUM operands in one instruction.)
                    tqv = tmp.tile([128, 512], F32, tag="tqv",
                                   name=f"tqv{g}_{sc}")
                    nc.vector.tensor_scalar_add(tqv, pq, bqk_t[g])
                    vts = tmp.tile([128, 512], BF16, tag="vts",
                                   name=f"vts{g}_{sc}")
                    nc.vector.tensor_add(vts, tqv, pk)
                    # softplus part 1: exp(k + bk) into one per-group tile;
                    # the single whole-tile Ln below cannot be interleaved by
                    # the scheduler, so the ACT table set switches only twice
                    # per group
                    nc.scalar.activation(out=tep[g][:, sc * 512:(sc + 1) * 512],
                                         in_=pk, func=AF.Exp, bias=bkb_t[g])
                    # q, scaled for the exp trick (window chunks only); on
                    # DVE — an ACT Identity would thrash the ACT table set
                    for wci, (wb, w) in enumerate(WCS):
                        if wb // 512 != sc:
                            continue
                        lo = wb % 512
                        nc.vector.tensor_scalar(qtp[g][wci][0:64, 0, :],
                                                pq[0:64, lo:lo + w], C16,
                                                bqs_t[g][0:64],
                                                ALU.mult, ALU.add)
                        nc.vector.tensor_scalar(qtp[g][wci][64:128, 1, :],
                                                pq[64:128, lo:lo + w], C16,
                                                bqs_t[g][64:128],
                                                ALU.mult, ALU.add)
                    vts_hist.append(vts)
                    if sc > 0:
                        emit_vtrans(g, sc - 1, vts_hist[sc - 1])
                    if sc == SC - 1:
                        emit_vtrans(g, sc, vts_hist[sc])
                nc.scalar.activation(out=kt[g], in_=tep[g], func=AF.Ln,
                                     bias=1.0)

            # exp engine rotation (Pool cannot read PSUM): ACT 60%, DVE 40%
            _exp_i = [0]
            _pat = ("act", "dve", "act", "act", "dve")

            def emit_exp(e_out, s_in):
                kind = _pat[_exp_i[0] % len(_pat)]
                _exp_i[0] += 1
                if kind == "act":
                    nc.scalar.activation(out=e_out, in_=s_in, func=AF.Exp,
                                         scale=EXP_SCALE, bias=shift_t)
                else:
                    nc.vector.tensor_scalar(e_out.bitcast(U16), s_in, K16, 0.0,
                                            ALU.add, ALU.max)

            def attn_group(g):
                vpA = vp[g * 2]
                vpB = vp[g * 2 + 1]
                # vsum (uniform masked-query row): ones-matmul over the
                # transposed v tiles, summed across keys on the PE
                for hh in range(2):
                    h = g * 2 + hh
                    pvs = ps_c.tile([65, 1], F32, tag=("cA" if hh == 0
                                                       else "cB"),
                                    name=f"pvs{g}_{hh}")
                    for kc in range(KC):
                        nc.tensor.matmul(pvs, vp[h][kc], ones_t,
                                         start=(kc == 0), stop=(kc == KC - 1))
                    vs_sb = ep.tile([64, 1], F32, tag="vss",
                                    name=f"vss{g}_{hh}")
                    nc.vector.tensor_copy(vs_sb, pvs[0:64, :])
                    row = g * 128 + hh * 64
                    nc.sync.dma_start(out=vout[row:row + 64, :], in_=vs_sb)
                for wi, (wb, w) in enumerate(WCS):
                    qAB = qtp[g][wi]
                    cA = ps_c.tile([65, w], F32, tag="cA", name=f"cA{g}_{wi}")
                    cB = ps_c.tile([65, w], F32, tag="cB", name=f"cB{g}_{wi}")
                    for kc in range(KC):
                        lhs = kt[g][:, kc * 128:(kc + 1) * 128]
                        sS = ps_s.tile([128, 2, w], F32, tag="s",
                                       name=f"sS{g}_{wi}_{kc}")
                        nc.tensor.matmul(sS, lhs, qAB, start=True, stop=True)
                        eS = expp.tile([128, 2, w], BF16, tag="eA",
                                       name=f"eS{g}_{wi}_{kc}")
                        emit_exp(eS, sS)
                        nc.tensor.matmul(cA, vpA[kc], eS[:, 0, :],
                                         start=(kc == 0), stop=(kc == KC - 1))
                        nc.tensor.matmul(cB, vpB[kc], eS[:, 1, :],
                                         start=(kc == 0), stop=(kc == KC - 1))
                    # epilogue: transpose back, normalize, store
                    csA = ep.tile([65, w], F32, tag="csA", name=f"csA{g}_{wi}")
                    nc.vector.tensor_copy(csA, cA)
                    csB = ep.tile([65, w], F32, tag="csB", name=f"csB{g}_{wi}")
                    nc.vector.tensor_copy(csB, cB)
                    jb = 0
                    while jb < w:
                        jw = min(128, w - jb)
                        ptA = ps_m.tile([jw, 65], F32, tag="ep",
                                        name=f"ptA{g}_{wi}_{jb}")
                        nc.tensor.transpose(ptA, csA[:, jb:jb + jw],
                                            ident_f)
                        ptB = ps_m.tile([jw, 65], F32, tag="ep",
                                        name=f"ptB{g}_{wi}_{jb}")
                        nc.tensor.transpose(ptB, csB[:, jb:jb + jw],
                                            ident_f)
                        rA = ep.tile([jw, 1], F32, tag="rA",
                                     name=f"rA{g}_{wi}_{jb}")
                        nc.vector.reciprocal(rA, ptA[:, 64:65])
                        rB = ep.tile([jw, 1], F32, tag="rB",
                                     name=f"rB{g}_{wi}_{jb}")
                        nc.vector.reciprocal(rB, ptB[:, 64:65])
                        cf = ep.tile([jw, 128], F32, tag="cf",
                                     name=f"cf{g}_{wi}_{jb}")
                        nc.vector.tensor_scalar_mul(cf[:, 0:64], ptA[:, 0:64], rA)
                        nc.vector.tensor_scalar_mul(cf[:, 64:128], ptB[:, 0:64],
                                                    rB)
                        row = wb + jb
                        nc.gpsimd.dma_start(
                            out=out[row:row + jw, g * 128:(g + 1) * 128],
                            in_=cf)
                        jb += jw

            for g in range(NG):
                proj_group(g)
                attn_group(g)

    nc.finalize()
    return nc


def _get_nc(NQ):
    key = ("nc", NQ)
    if key not in _CACHE:
        _CACHE[key] = _build(NQ)
    return _CACHE[key]


def _shard_inputs(hidden_states, attention_mask, Wq, bq, Wk, bk):
    import ml_dtypes
    BF = ml_dtypes.bfloat16

    hs = np.asarray(hidden_states, dtype=np.float32)
    am = np.asarray(attention_mask)
    Wq = np.asarray(Wq, dtype=np.float32)
    Wk = np.asarray(Wk, dtype=np.float32)
    bq = np.asarray(bq, dtype=np.float32)
    bk = np.asarray(bk, dtype=np.float32)

    perms, n_ums = [], []
    for b in range(B):
        perm = np.argsort(-(am[b] != 0).astype(np.int32), kind="stable")
        perms.append(perm)
        n_ums.append(int((am[b] != 0).sum()))
    NQ = max(128, -(-max(n_ums) // 64) * 64)
    NQ = min(NQ, S)
    _CACHE["meta"] = {"perms": perms, "n_ums": n_ums, "NQ": NQ}

    xts = []
    for b in range(B):
        xp = hs[b][perms[b]]
        xts.append(np.ascontiguousarray(
            xp.T.reshape(8, 128, SC, 512).transpose(2, 0, 1, 3)
            .reshape(SC * 8 * 128, 512)).astype(BF).view(np.float32))

    in_maps = []
    for c in range(NCORES):
        b = c // (NCORES // B)
        hg = c % (NCORES // B)
        cols = slice(hg * 2 * 128, (hg + 1) * 2 * 128)

        def _tile_w(W):
            return np.ascontiguousarray(
                W[:, cols].reshape(8, 128, NG, 128).transpose(2, 0, 1, 3)
                .reshape(NG * 8 * 128, 128)).astype(BF).view(np.float32)

        in_maps.append({
            "xt": xts[b],
            "wq": _tile_w(Wq),
            "wk": _tile_w(Wk),
            "bqs": np.ascontiguousarray(C16 * bq[cols]).astype(np.float32),
            "bqk": np.ascontiguousarray(bq[cols] + bk[cols]).astype(np.float32),
            "bkb": np.ascontiguousarray(bk[cols]).astype(np.float32),
        })
    return in_maps


def _gather(results):
    meta = _CACHE["meta"]
    perms, n_ums, NQ = meta["perms"], meta["n_ums"], meta["NQ"]
    full = np.empty((B, S, D), dtype=np.float32)
    for c in range(NCORES):
        b = c // (NCORES // B)
        hg = c % (NCORES // B)
        cols = slice(hg * 2 * 128, (hg + 1) * 2 * 128)
        permuted = np.empty((S, 2 * 128), dtype=np.float32)
        permuted[:NQ] = results[c]["out"]
        permuted[n_ums[b]:] = (results[c]["vout"][:, 0] / np.float32(S))[None, :]
        full[b, :, cols] = permuted[np.argsort(perms[b])]
    return full


def run_sharded(in_maps, **kw):
    from concourse.bass_utils import run_bass_kernel_spmd
    nc = _get_nc(_CACHE["meta"]["NQ"])
    return run_bass_kernel_spmd(nc, in_maps, list(range(NCORES)), **kw)


def kernel(hidden_states, attention_mask, Wq, bq, Wk, bk):
    in_maps = _shard_inputs(hidden_states, attention_mask, Wq, bq, Wk, bk)
    res = run_sharded(in_maps)
    return _gather(res.results)
